# revision 34
# baseline (speedup 1.0000x reference)
"""Expert-parallel MoE MLP (top-2 of 8 experts) on 8 TRN2 NeuronCores.

Strategy (expert-parallel, per sharding hint):
  - core e holds expert e's weights (w1[e], w2[e], host-pre-transposed, bf16)
  - host dispatches tokens by expert id; compute runs over a COMPACT column
    set (ctok = max tokens per expert, padded to 128); padding columns
    beyond the real token count are skipped in mm1
  - mm1 emitted in 2-tile (256-col) segments interleaved with mm2 128-token
    tiles. The first two segments use k-SPLIT accumulation: matmuls over
    k 0:4 start once HALF of w1 has landed (~13us), partials drain to SBUF
    on the idle Vector engine, and the k 4:8 phase adds them back while the
    rest of w1 streams in (the load fabric supplies only ~45 GB/s per
    issuing queue, so w1 is not fully resident until ~40us)
  - each mm2 tile's [128, 1024] result is scaled by the combine weights and
    indirect-DMA-scattered into block-padded per-chunk send buffers,
    column-split via element_offset so the pieces stream on parallel DMA
    queues (partition-base>0 indirect operands crash the DMA ucode)
  - the A2A is split into up to 4 slot-range chunks on 16-slot boundaries,
    sized adaptively so they trigger after tiles [n/2, n-2, n-1, n]: the
    last chunk is small (only the deepest slots) so the post-compute tail
    is one short collective instead of a third of the payload. Chunk-firing
    scatters go first in their tile with the doorbell right behind them;
    combine gathers are dep-gated behind the NEXT chunk's trigger so a
    data-blocked gather never head-blocks a doorbell
  - a tiny warmup AllToAll at program start absorbs the ~50us first-op
    ring-arming barrier; real chunks then run at ~3.6us + 9us/MB + peer
    start-skew (~10-25us of runtime dispatch jitter)
  - send-buffer padding rows are never zeroed: the A2A ships garbage in
    them but the combine gathers only reference real rows
  - combine: owner tokens are host-sorted by the chunk their later partial
    row lands in, so early combine tiles (64 tokens) overlap later A2A
    chunks; per-row classes gate each gather separately; single-expert
    tokens gather the same row twice with a halved combine weight; output
    is written bf16 (host upcasts) in two column halves on two queues;
    host unpermutes the rows
"""

import sys

sys.path.insert(0, "/opt/trn_rl_repo")

import numpy as np
import ml_dtypes

import concourse.bass as bass
import concourse.tile as tile
from concourse import bacc, mybir
from concourse.bass_utils import run_bass_kernel_spmd
from concourse.tile_rust import add_dep_helper

S, DM, DF, E, TOPK = 4096, 1024, 2048, 8, 2
NCORES = 8
P = 128
OWN = S // NCORES  # tokens per owner core
CB = 64  # combine tile rows
SPLIT_SCATTER = True  # column-halves via element_offset (partition bases 0)
SPLIT_GATHER = True
WARMUP_A2A = True
ACT = mybir.ActivationFunctionType.Silu

_PROGRAM_CACHE: dict = {}


def _emit(nc, tc, ctx, cfg):
    blk, B, ctok, tr, Ts, comb_cls0, comb_cls1, gsets = cfg[:8]
    dt = mybir.dt
    nch = len(B) - 1
    ntiles = ctok // P
    sg = [B[g + 1] - B[g] for g in range(nch)]
    Rg = [NCORES * b for b in B]  # chunk row starts in recvbuf

    xT = nc.dram_tensor("xT", [DM, ctok], dt.bfloat16, kind="ExternalInput").ap()
    w1t = nc.dram_tensor("w1t", [DM, DF], dt.bfloat16, kind="ExternalInput").ap()
    w2t = nc.dram_tensor("w2t", [DF, DM], dt.bfloat16, kind="ExternalInput").ap()
    cv = nc.dram_tensor("cv", [ctok], dt.float32, kind="ExternalInput").ap()
    rscs = [
        nc.dram_tensor(f"rsc{g}", [ctok], dt.int32, kind="ExternalInput").ap()
        for g in range(nch)
    ]
    g0 = nc.dram_tensor("g0", [OWN], dt.int32, kind="ExternalInput").ap()
    g1 = nc.dram_tensor("g1", [OWN], dt.int32, kind="ExternalInput").ap()
    # bf16 output halves the tail write bytes; host upcasts to fp32 (the
    # final sum rounds once to bf16: ~1e-3 extra rel err vs the 2e-2 gate)
    yout = nc.dram_tensor("yout", [OWN, DM], dt.bfloat16, kind="ExternalOutput").ap()
    # per-chunk send tensors, each with +1 garbage row (scatter APs must be
    # offset-0; separate tensors keep chunk deps disjoint)
    sends = [
        nc.dram_tensor(f"send{g}", [NCORES * sg[g] + 1, DM], dt.bfloat16).ap()
        for g in range(nch)
    ]
    recvbuf = nc.dram_tensor("recvbuf", [NCORES * blk, DM], dt.bfloat16).ap()

    wpool = ctx.enter_context(tc.tile_pool(name="w", bufs=1))
    hpool = ctx.enter_context(tc.tile_pool(name="h", bufs=2))
    ypool = ctx.enter_context(tc.tile_pool(name="y", bufs=8))
    gpool = ctx.enter_context(tc.tile_pool(name="g", bufs=3))
    phpool = ctx.enter_context(tc.tile_pool(name="ph", bufs=3, space="PSUM"))
    pypool = ctx.enter_context(tc.tile_pool(name="py", bufs=5, space="PSUM"))

    groups = [list(range(NCORES))]

    if WARMUP_A2A:
        dummy_s = nc.dram_tensor("dummy_s", [NCORES, 32], dt.bfloat16).ap()
        dummy_r = nc.dram_tensor("dummy_r", [NCORES, 32], dt.bfloat16).ap()
        zrow = wpool.tile([NCORES, 32], dt.bfloat16, tag="zrow")
        nc.vector.memset(zrow[:], 0.0)
        nc.sync.dma_start(dummy_s[:, :], zrow[:])
        nc.gpsimd.collective_compute(
            "AllToAll",
            mybir.AluOpType.bypass,
            replica_groups=groups,
            ins=[dummy_s],
            outs=[dummy_r],
        )

    # ---- loads: ~128KB pieces alternating over the two fast issue queues -
    # each dma_start lands on one HW DMA queue (~23 GB/s); parallel
    # bandwidth comes from MANY concurrent in-flight DMAs. Pieces are
    # issued in consumption order for the k-split mm1 start: x/w1 for
    # k 0:4 first (phase-A matmuls begin ~3us after the preamble), then
    # k 4:8, then w2 f-ascending. Late-needed pieces go on gpsimd.
    w1sb = wpool.tile([P, DM // P, DF], dt.bfloat16, tag="w1sb")
    w1r = w1t.rearrange("(o p) f -> p o f", p=P)
    xsb = wpool.tile([P, DM // P, ctok], dt.bfloat16, tag="xsb")
    xr = xT.rearrange("(o p) t -> p o t", p=P)
    w2sb = wpool.tile([P, DF // P, DM], dt.bfloat16, tag="w2sb")
    w2r = w2t.rearrange("(o p) d -> p o d", p=P)
    ldq = [nc.sync, nc.scalar]
    lqi = [0]

    def issue(dst, src):
        ldq[lqi[0] % len(ldq)].dma_start(dst, src)
        lqi[0] += 1

    c1 = min(512, ctok)
    ch = min(256, ctok)
    kh = DM // P // 2
    for k in range(kh):
        issue(xsb[:, k, 0:ch], xr[:, k, 0:ch])
    for fq in range(4):
        f0, f1 = fq * (DF // 4), (fq + 1) * (DF // 4)
        for k in range(kh):
            issue(w1sb[:, k, f0:f1], w1r[:, k, f0:f1])
        if fq == 0 and c1 > ch:
            for k in range(kh):
                issue(xsb[:, k, ch:c1], xr[:, k, ch:c1])
    for k in range(kh, DM // P):
        issue(xsb[:, k, 0:c1], xr[:, k, 0:c1])
    for fq in range(4):
        f0, f1 = fq * (DF // 4), (fq + 1) * (DF // 4)
        for k in range(kh, DM // P):
            issue(w1sb[:, k, f0:f1], w1r[:, k, f0:f1])
    issue(w2sb[:, 0:2, :], w2r[:, 0:2, :])
    for f0 in range(2, 8, 2):
        issue(w2sb[:, f0 : f0 + 2, :], w2r[:, f0 : f0 + 2, :])
    if ctok > c1:
        for k0 in range(0, DM // P, 2):
            nc.gpsimd.dma_start(
                xsb[:, k0 : k0 + 2, c1:ctok], xr[:, k0 : k0 + 2, c1:ctok]
            )
    csb = wpool.tile([P, ntiles], dt.float32, tag="csb")
    nc.gpsimd.dma_start(csb[:], cv.rearrange("(t p) -> p t", p=P))
    rssbs = []
    for g in range(nch):
        rssb = wpool.tile([P, ntiles], dt.int32, tag=f"rssb{g}", name=f"rssb{g}")
        nc.gpsimd.dma_start(rssb[:], rscs[g].rearrange("(t p) -> p t", p=P))
        rssbs.append(rssb)
    for f0 in range(8, DF // P, 2):
        nc.gpsimd.dma_start(w2sb[:, f0 : f0 + 2, :], w2r[:, f0 : f0 + 2, :])
    g0sb = wpool.tile([CB, OWN // CB], dt.int32, tag="g0sb")
    nc.gpsimd.dma_start(g0sb[:], g0.rearrange("(t p) -> p t", p=CB))
    g1sb = wpool.tile([CB, OWN // CB], dt.int32, tag="g1sb")
    nc.gpsimd.dma_start(g1sb[:], g1.rearrange("(t p) -> p t", p=CB))

    # ---- interleaved mm1 (2-tile segments) / mm2 (token tiles) ----------
    # mm1 segments are 256 cols: wide enough that ldweights stays hidden
    # under the moving dim, narrow enough that mm2 tiles start early
    segs = []
    t = 0
    while t < ntiles:
        b = min(t + 2, ntiles)
        segs.append((t, b))
        t = b

    fired = [False] * nch
    trig = [None] * nch
    hs = None
    hbase = 0
    done_mm1 = 0
    si = 0

    # k-split phase A for the first KS segments: matmuls over k 0:4 start
    # as soon as HALF of w1 has landed (~12us instead of ~28), drained to
    # SBUF partials on the (idle) Vector engine; phase B adds k 4:8 and
    # runs while the second half of w1 streams in
    KS = len(segs)
    kh_ = DM // P // 2
    hA = {}
    hapool = ctx.enter_context(tc.tile_pool(name="ha", bufs=1))
    hspool = ctx.enter_context(tc.tile_pool(name="hsum", bufs=3))
    for s in range(KS):
        a, b = segs[s]
        c0, csz = a * P, (b - a) * P
        csz_r = min(csz, tr - c0)
        hA[s] = [
            hapool.tile(
                [P, csz_r], dt.bfloat16, tag=f"hA{s}_{i}", name=f"hA{s}_{i}"
            )
            for i in range(DF // P)
        ]
        for i in range(DF // P):
            ph = phpool.tile([P, csz_r], dt.float32, tag="ph")
            for k in range(kh_):
                nc.tensor.matmul(
                    ph[:],
                    lhsT=w1sb[:, k, i * P : (i + 1) * P],
                    rhs=xsb[:, k, c0 : c0 + csz_r],
                    start=(k == 0),
                    stop=(k == kh_ - 1),
                )
            nc.vector.tensor_scalar_add(hA[s][i][:], ph[:], 0.0)

    def emit_mm1():
        nonlocal hs, hbase, done_mm1, si
        s = si
        a, b = segs[si]
        si += 1
        c0, csz = a * P, (b - a) * P
        # skip the padding columns beyond the real token count: mm2 reads
        # the stale hs region for pad tokens, whose y rows scatter to the
        # garbage row anyway
        csz_r = min(csz, tr - c0)
        hbase = c0
        done_mm1 = c0 + csz
        hs = [
            hpool.tile([P, csz], dt.bfloat16, tag=f"h{i}", name=f"h{i}")
            for i in range(DF // P)
        ]
        k0_ = kh_ if s < KS else 0
        for i in range(DF // P):
            ph = phpool.tile([P, csz_r], dt.float32, tag="ph")
            for k in range(k0_, DM // P):
                nc.tensor.matmul(
                    ph[:],
                    lhsT=w1sb[:, k, i * P : (i + 1) * P],
                    rhs=xsb[:, k, c0 : c0 + csz_r],
                    start=(k == k0_),
                    stop=(k == DM // P - 1),
                )
            if s < KS:
                hsum = hspool.tile([P, csz_r], dt.bfloat16, tag="hsum", name="hsum")
                nc.vector.tensor_add(hsum[:], ph[:], hA[s][i][:])
                nc.scalar.activation(hs[i][:, 0:csz_r], hsum[:], ACT)
            else:
                nc.scalar.activation(hs[i][:, 0:csz_r], ph[:], ACT)

    for tm in range(ntiles):
        while (tm + 1) * P > done_mm1:
            emit_mm1()
        toff = tm * P - hbase
        py0 = pypool.tile([P, 512], dt.float32, tag="py")
        py1 = pypool.tile([P, 512], dt.float32, tag="py")
        for f in range(DF // P):
            lhs = hs[f][:, toff : toff + P]
            nc.tensor.matmul(
                py0[:], lhsT=lhs, rhs=w2sb[:, f, 0:512],
                start=(f == 0), stop=(f == DF // P - 1),
            )
            nc.tensor.matmul(
                py1[:], lhsT=lhs, rhs=w2sb[:, f, 512:1024],
                start=(f == 0), stop=(f == DF // P - 1),
            )
        # scale on the (otherwise idle) Scalar engine: keeping these off the
        # Vector queue stops the combine adds' gather-waits from head-blocking
        # the PSUM release chain
        y_sb = ypool.tile([P, DM], dt.bfloat16, tag="y")
        nc.scalar.activation(
            y_sb[:, 0:512], py0[:],
            mybir.ActivationFunctionType.Copy, scale=csb[:, tm : tm + 1],
        )
        nc.scalar.activation(
            y_sb[:, 512:1024], py1[:],
            mybir.ActivationFunctionType.Copy, scale=csb[:, tm : tm + 1],
        )
        # scatter only the real rows (pad rows would land on the garbage
        # row anyway). Split by COLUMN ranges via element_offset -- pieces
        # stream on separate DMA queues (~23 GB/s each), cutting the
        # y->send latency between the last mm2 tile and its A2A doorbell.
        # (All partition bases stay 0: base>0 indirect operands crash the
        # DMA ucode.)
        rows = min(P, tr - tm * P)
        # gating chunks (whose A2A fires after this tile) scatter FIRST
        # and trigger immediately; other chunks' scatters queue after the
        # doorbell so they don't delay it
        for g in sorted(gsets[tm], key=lambda g_: Ts[g_] != tm + 1):
            gating = Ts[g] == tm + 1
            nsp = 2 if SPLIT_SCATTER else 1
            splits = [(i * DM // nsp, (i + 1) * DM // nsp) for i in range(nsp)]
            for e0, e1 in splits:
                last_scat = nc.gpsimd.indirect_dma_start(
                    out=sends[g][:],
                    out_offset=bass.IndirectOffsetOnAxis(
                        ap=rssbs[g][0:rows, tm : tm + 1], axis=0
                    ),
                    in_=y_sb[0:rows, e0:e1],
                    in_offset=None,
                    element_offset=e0,
                )
            if gating and not fired[g]:
                trig[g] = nc.gpsimd.collective_compute(
                    "AllToAll",
                    mybir.AluOpType.bypass,
                    replica_groups=groups,
                    ins=[sends[g][0 : NCORES * sg[g], :]],
                    outs=[recvbuf[Rg[g] : Rg[g + 1], :]],
                )
                fired[g] = True
        for g in range(nch):
            if not fired[g] and Ts[g] == tm + 1:
                trig[g] = nc.gpsimd.collective_compute(
                    "AllToAll",
                    mybir.AluOpType.bypass,
                    replica_groups=groups,
                    ins=[sends[g][0 : NCORES * sg[g], :]],
                    outs=[recvbuf[Rg[g] : Rg[g + 1], :]],
                )
                fired[g] = True
    assert all(fired), (Ts, ntiles)

    # ---- combine: class-sorted 64-token tiles, prefix-sliced gathers ----
    # per-token rows are host-ordered (g0 = earlier-landing chunk, g1 =
    # later), each gather gated/prefixed by its own class so the early-row
    # gather overlaps the last A2A chunk instead of waiting for it
    for j in range(OWN // CB):
        last_cls = comb_cls1[j] == nch - 1
        ga = gpool.tile([CB, DM], dt.bfloat16, tag="ga")
        gb = gpool.tile([CB, DM], dt.bfloat16, tag="gb")
        gis = []
        # last-class gathers are on the post-final-A2A critical path: split
        # them by column half (element_offset) so the two 64KB pieces
        # stream on two DMA queues
        halves = (
            ((0, 512), (512, DM)) if (last_cls and SPLIT_GATHER) else ((0, DM),)
        )
        for buf, gsb, cls in ((ga, g0sb, comb_cls0[j]), (gb, g1sb, comb_cls1[j])):
            for e0, e1 in halves:
                gi = nc.gpsimd.indirect_dma_start(
                    out=buf[:, e0:e1],
                    out_offset=None,
                    in_=recvbuf[0 : Rg[cls + 1], :],
                    in_offset=bass.IndirectOffsetOnAxis(
                        ap=gsb[:, j : j + 1], axis=0
                    ),
                    element_offset=e0,
                )
                gis.append((gi, cls))
        # queue class-c gathers behind the DOORBELL of chunk c+1: the
        # scheduler otherwise slots them between scatters and a trigger,
        # where their (blocked) recvbuf wait head-blocks the gpsimd queue
        # and delays the doorbell until the previous chunk lands. Gating on
        # c+1's trigger still lets early-class gathers overlap later chunks.
        for gi, c in gis:
            gate = trig[min(c + 1, nch - 1)]
            add_dep_helper(gi.ins, gate.ins, sync=False, reason="doorbells first")
        ys = gpool.tile([CB, DM], dt.bfloat16, tag="ys")
        nc.vector.tensor_add(ys[:], ga[:], gb[:])
        # split the output write across two issue queues (two DMA queues)
        nc.sync.dma_start(yout[j * CB : j * CB + CB // 2, :], ys[0 : CB // 2, :])
        nc.scalar.dma_start(yout[j * CB + CB // 2 : (j + 1) * CB, :], ys[CB // 2 : CB, :])


def _build_program(cfg):
    key = cfg[:8]
    if key in _PROGRAM_CACHE:
        return _PROGRAM_CACHE[key]
    from contextlib import ExitStack

    nc = bacc.Bacc(
        "TRN2",
        target_bir_lowering=False,
        debug=False,
        enable_asserts=True,
        num_devices=NCORES,
    )
    with tile.TileContext(nc) as tc:
        with ExitStack() as ctx:
            _emit(nc, tc, ctx, cfg)
    nc.compile()
    _PROGRAM_CACHE[key] = nc
    return nc


def _prepare(x, topk_e, topk_w):
    """Host-side routing: dispatch tokens to experts.

    Layout: token with slot s in (expert e -> owner d) block, s in
    [B_g, B_{g+1}):  send-side row (core e, tensor send_g) = d*sg + (s-B_g);
    recv-side row (core d, recvbuf) = 8*B_g + e*sg + (s-B_g).
    """
    bf16 = ml_dtypes.bfloat16
    c = np.zeros((S, E), dtype=np.float32)
    np.add.at(c, (np.arange(S)[:, None], topk_e), topk_w.astype(np.float32))
    single = topk_e[:, 0] == topk_e[:, 1]

    toks = [np.nonzero((topk_e == e).any(axis=1))[0] for e in range(E)]
    cnt = np.zeros((E, NCORES), dtype=np.int64)
    for e in range(E):
        cnt[e] = np.bincount(toks[e] // OWN, minlength=NCORES)
    blk = int(cnt.max())

    ntok_max = max(len(t) for t in toks)
    ctok = int(-(-ntok_max // P) * P)
    ntiles = ctok // P

    # Adaptive chunk boundaries: chunk g should trigger after tile
    # targets[g], i.e. the largest 16-multiple slot b such that every
    # expert's token count with slot < b fits in targets[g]*128 rows.
    # Late triggers get small chunks so the post-compute tail is short.
    def maxcum(b):
        return int(np.minimum(cnt, b).sum(axis=1).max())

    targets = [ntiles // 2, ntiles - 2, ntiles - 1]
    B = [0]
    for tgt in targets:
        b = B[-1]
        nb = b
        while nb + 16 < blk and maxcum(nb + 16) <= tgt * P:
            nb += 16
        if nb > b:
            B.append(nb)
    B.append(blk)
    B = sorted(set(B))
    nch = len(B) - 1
    Ba = np.array(B)
    sga = np.diff(Ba)
    Rga = NCORES * Ba

    in_maps = []
    row_of = {}  # (e, token) -> absolute recvbuf row (on the owner core)
    cums = np.zeros((E, nch), dtype=np.int64)
    gsets = [set() for _ in range(ntiles)]
    for e in range(E):
        te = toks[e]
        d = te // OWN
        seg_start = np.searchsorted(te, np.arange(NCORES) * OWN)
        slot = np.arange(len(te)) - seg_start[d]
        gi = np.searchsorted(Ba[1:-1], slot, side="right")
        srow = d * sga[gi] + (slot - Ba[gi])  # send side, relative to send_g
        rrow = Rga[gi] + e * sga[gi] + (slot - Ba[gi])  # recv side, absolute
        for t, r in zip(te, rrow):
            row_of[(e, int(t))] = int(r)
        order = np.lexsort((slot, d, gi))
        te_o = te[order]
        gi_o = np.full(ctok, nch - 1, dtype=np.int64)
        gi_o[: len(te)] = gi[order]
        xT_e = np.zeros((DM, ctok), dtype=bf16)
        xT_e[:, : len(te)] = x[te_o].T.astype(bf16)
        cv_e = np.zeros(ctok, dtype=np.float32)
        w = c[te_o, e]
        cv_e[: len(te)] = np.where(single[te_o], 0.5 * w, w)
        im = {"xT": xT_e, "cv": cv_e}
        srow_o = np.zeros(ctok, dtype=np.int64)
        srow_o[: len(te)] = srow[order]
        for g in range(nch):
            rs = np.full(ctok, NCORES * sga[g], dtype=np.int32)  # garbage row
            sel = gi_o == g
            sel[len(te) :] = False
            rs[sel] = srow_o[sel]
            im[f"rsc{g}"] = rs
            cums[e, g] = int(np.sum(gi <= g))
        for tm in range(ntiles):
            for g in np.unique(gi_o[tm * P : (tm + 1) * P]):
                gsets[tm].add(int(g))
        in_maps.append(im)

    Ts = [min(int(np.ceil(cums[:, g].max() / P)), ntiles) for g in range(nch)]
    for g in range(1, nch):
        Ts[g] = max(Ts[g], Ts[g - 1])
    Ts[-1] = ntiles

    chunk_of_row = lambda r: int(np.searchsorted(Rga[1:], r, side="right"))
    perms = []
    comb_cls0 = np.zeros((NCORES, OWN // CB), dtype=np.int64)
    comb_cls1 = np.zeros((NCORES, OWN // CB), dtype=np.int64)
    for dcore in range(NCORES):
        r0a = np.zeros(OWN, dtype=np.int32)
        r1a = np.zeros(OWN, dtype=np.int32)
        cls0 = np.zeros(OWN, dtype=np.int64)
        cls1 = np.zeros(OWN, dtype=np.int64)
        for t_loc in range(OWN):
            t = dcore * OWN + t_loc
            es = np.unique(topk_e[t])
            ra = row_of[(int(es[0]), t)]
            rb = row_of[(int(es[1]), t)] if len(es) > 1 else ra
            ca, cb = chunk_of_row(ra), chunk_of_row(rb)
            if ca > cb:
                ra, rb, ca, cb = rb, ra, cb, ca
            r0a[t_loc], r1a[t_loc] = ra, rb
            cls0[t_loc], cls1[t_loc] = ca, cb
        perm = np.argsort(cls1, kind="stable")
        perms.append(perm)
        comb_cls0[dcore] = cls0[perm].reshape(OWN // CB, CB).max(axis=1)
        comb_cls1[dcore] = cls1[perm].reshape(OWN // CB, CB).max(axis=1)
        in_maps[dcore]["g0"] = r0a[perm]
        in_maps[dcore]["g1"] = r1a[perm]

    cfg = (
        blk,
        tuple(B),
        ctok,
        ntok_max,
        tuple(Ts),
        tuple(int(v) for v in comb_cls0.max(axis=0)),
        tuple(int(v) for v in comb_cls1.max(axis=0)),
        tuple(tuple(sorted(s)) for s in gsets),
        tuple(tuple(int(v) for v in p) for p in perms),
    )
    return in_maps, cfg


def prepare_in_maps(x, topk_e, topk_w, w1, w2):
    bf16 = ml_dtypes.bfloat16
    in_maps, cfg = _prepare(np.asarray(x), np.asarray(topk_e), np.asarray(topk_w))
    for e in range(E):
        in_maps[e]["w1t"] = np.ascontiguousarray(np.asarray(w1)[e].T).astype(bf16)
        in_maps[e]["w2t"] = np.ascontiguousarray(np.asarray(w2)[e].T).astype(bf16)
    return in_maps, cfg


def postprocess(results, cfg):
    perms = cfg[8]
    out = np.empty((S, DM), dtype=np.float32)
    for d in range(NCORES):
        out[d * OWN + np.asarray(perms[d], dtype=np.int64)] = results[d][
            "yout"
        ].astype(np.float32)
    return out


def kernel(x, topk_e, topk_w, w1, w2):
    in_maps, cfg = prepare_in_maps(x, topk_e, topk_w, w1, w2)
    nc = _build_program(cfg)
    res = run_bass_kernel_spmd(nc, in_maps, list(range(NCORES)))
    return postprocess(res.results, cfg)


# revision 35
# speedup vs baseline: 1.0241x; 1.0241x over previous
"""Expert-parallel MoE MLP (top-2 of 8 experts) on 8 TRN2 NeuronCores.

Strategy (expert-parallel, per sharding hint):
  - core e holds expert e's weights (w1[e], w2[e], host-pre-transposed, bf16)
  - host dispatches tokens by expert id; compute runs over a COMPACT column
    set (ctok = max tokens per expert, padded to 128); padding columns
    beyond the real token count are skipped in mm1
  - mm1 emitted in 2-tile (256-col) segments interleaved with mm2 128-token
    tiles. The first two segments use k-SPLIT accumulation: matmuls over
    k 0:4 start once HALF of w1 has landed (~13us), partials drain to SBUF
    on the idle Vector engine, and the k 4:8 phase adds them back while the
    rest of w1 streams in (the load fabric supplies only ~45 GB/s per
    issuing queue, so w1 is not fully resident until ~40us)
  - each mm2 tile's [128, 1024] result is scaled by the combine weights and
    indirect-DMA-scattered into block-padded per-chunk send buffers,
    column-split via element_offset so the pieces stream on parallel DMA
    queues (partition-base>0 indirect operands crash the DMA ucode)
  - the A2A is split into up to 4 slot-range chunks on 16-slot boundaries,
    sized adaptively so they trigger after tiles [n/2, n-2, n-1, n]: the
    last chunk is small (only the deepest slots) so the post-compute tail
    is one short collective instead of a third of the payload. Chunk-firing
    scatters go first in their tile with the doorbell right behind them;
    combine gathers are dep-gated behind the NEXT chunk's trigger so a
    data-blocked gather never head-blocks a doorbell
  - a tiny warmup AllToAll at program start absorbs the ~50us first-op
    ring-arming barrier; real chunks then run at ~3.6us + 9us/MB + peer
    start-skew (~10-25us of runtime dispatch jitter)
  - send-buffer padding rows are never zeroed: the A2A ships garbage in
    them but the combine gathers only reference real rows
  - combine: owner tokens are host-sorted by the chunk their later partial
    row lands in, so early combine tiles (64 tokens) overlap later A2A
    chunks; per-row classes gate each gather separately; single-expert
    tokens gather the same row twice with a halved combine weight; output
    is written bf16 (host upcasts) in two column halves on two queues;
    host unpermutes the rows
"""

import sys

sys.path.insert(0, "/opt/trn_rl_repo")

import numpy as np
import ml_dtypes

import concourse.bass as bass
import concourse.tile as tile
from concourse import bacc, mybir
from concourse.bass_utils import run_bass_kernel_spmd
from concourse.tile_rust import add_dep_helper

S, DM, DF, E, TOPK = 4096, 1024, 2048, 8, 2
NCORES = 8
P = 128
OWN = S // NCORES  # tokens per owner core
CB = 64  # combine tile rows
SPLIT_SCATTER = True  # column-halves via element_offset (partition bases 0)
SPLIT_GATHER = True
WARMUP_A2A = True
ACT = mybir.ActivationFunctionType.Silu

_PROGRAM_CACHE: dict = {}


def _emit(nc, tc, ctx, cfg):
    blk, B, ctok, tr, Ts, comb_cls0, comb_cls1, gsets = cfg[:8]
    dt = mybir.dt
    nch = len(B) - 1
    ntiles = ctok // P
    sg = [B[g + 1] - B[g] for g in range(nch)]
    Rg = [NCORES * b for b in B]  # chunk row starts in recvbuf

    xT = nc.dram_tensor("xT", [DM, ctok], dt.bfloat16, kind="ExternalInput").ap()
    w1t = nc.dram_tensor("w1t", [DM, DF], dt.bfloat16, kind="ExternalInput").ap()
    w2t = nc.dram_tensor("w2t", [DF, DM], dt.bfloat16, kind="ExternalInput").ap()
    cv = nc.dram_tensor("cv", [ctok], dt.float32, kind="ExternalInput").ap()
    rscs = [
        nc.dram_tensor(f"rsc{g}", [ctok], dt.int32, kind="ExternalInput").ap()
        for g in range(nch)
    ]
    g0 = nc.dram_tensor("g0", [OWN], dt.int32, kind="ExternalInput").ap()
    g1 = nc.dram_tensor("g1", [OWN], dt.int32, kind="ExternalInput").ap()
    # bf16 output halves the tail write bytes; host upcasts to fp32 (the
    # final sum rounds once to bf16: ~1e-3 extra rel err vs the 2e-2 gate)
    yout = nc.dram_tensor("yout", [OWN, DM], dt.bfloat16, kind="ExternalOutput").ap()
    # per-chunk send tensors, each with +1 garbage row (scatter APs must be
    # offset-0; separate tensors keep chunk deps disjoint)
    sends = [
        nc.dram_tensor(f"send{g}", [NCORES * sg[g] + 1, DM], dt.bfloat16).ap()
        for g in range(nch)
    ]
    recvbuf = nc.dram_tensor("recvbuf", [NCORES * blk, DM], dt.bfloat16).ap()

    wpool = ctx.enter_context(tc.tile_pool(name="w", bufs=1))
    hpool = ctx.enter_context(tc.tile_pool(name="h", bufs=2))
    ypool = ctx.enter_context(tc.tile_pool(name="y", bufs=8))
    gpool = ctx.enter_context(tc.tile_pool(name="g", bufs=3))
    phpool = ctx.enter_context(tc.tile_pool(name="ph", bufs=3, space="PSUM"))
    pypool = ctx.enter_context(tc.tile_pool(name="py", bufs=5, space="PSUM"))

    groups = [list(range(NCORES))]

    if WARMUP_A2A:
        dummy_s = nc.dram_tensor("dummy_s", [NCORES, 32], dt.bfloat16).ap()
        dummy_r = nc.dram_tensor("dummy_r", [NCORES, 32], dt.bfloat16).ap()
        zrow = wpool.tile([NCORES, 32], dt.bfloat16, tag="zrow")
        nc.vector.memset(zrow[:], 0.0)
        nc.sync.dma_start(dummy_s[:, :], zrow[:])
        nc.gpsimd.collective_compute(
            "AllToAll",
            mybir.AluOpType.bypass,
            replica_groups=groups,
            ins=[dummy_s],
            outs=[dummy_r],
        )

    # ---- loads: ~128KB pieces alternating over the two fast issue queues -
    # each dma_start lands on one HW DMA queue (~23 GB/s); parallel
    # bandwidth comes from MANY concurrent in-flight DMAs. Pieces are
    # issued in consumption order for the k-split mm1 start: x/w1 for
    # k 0:4 first (phase-A matmuls begin ~3us after the preamble), then
    # k 4:8, then w2 f-ascending. Late-needed pieces go on gpsimd.
    w1sb = wpool.tile([P, DM // P, DF], dt.bfloat16, tag="w1sb")
    w1r = w1t.rearrange("(o p) f -> p o f", p=P)
    xsb = wpool.tile([P, DM // P, ctok], dt.bfloat16, tag="xsb")
    xr = xT.rearrange("(o p) t -> p o t", p=P)
    w2sb = wpool.tile([P, DF // P, DM], dt.bfloat16, tag="w2sb")
    w2r = w2t.rearrange("(o p) d -> p o d", p=P)
    ldq = [nc.sync, nc.scalar]
    lqi = [0]

    def issue(dst, src):
        ldq[lqi[0] % len(ldq)].dma_start(dst, src)
        lqi[0] += 1

    c1 = min(512, ctok)
    ch = min(256, ctok)
    kh = DM // P // 2
    for k in range(kh):
        issue(xsb[:, k, 0:ch], xr[:, k, 0:ch])
    for fq in range(4):
        f0, f1 = fq * (DF // 4), (fq + 1) * (DF // 4)
        for k in range(kh):
            issue(w1sb[:, k, f0:f1], w1r[:, k, f0:f1])
        if fq == 0 and c1 > ch:
            for k in range(kh):
                issue(xsb[:, k, ch:c1], xr[:, k, ch:c1])
    for k in range(kh, DM // P):
        issue(xsb[:, k, 0:c1], xr[:, k, 0:c1])
    for fq in range(4):
        f0, f1 = fq * (DF // 4), (fq + 1) * (DF // 4)
        for k in range(kh, DM // P):
            issue(w1sb[:, k, f0:f1], w1r[:, k, f0:f1])
    issue(w2sb[:, 0:2, :], w2r[:, 0:2, :])
    for f0 in range(2, 8, 2):
        issue(w2sb[:, f0 : f0 + 2, :], w2r[:, f0 : f0 + 2, :])
    if ctok > c1:
        for k0 in range(0, DM // P, 2):
            nc.gpsimd.dma_start(
                xsb[:, k0 : k0 + 2, c1:ctok], xr[:, k0 : k0 + 2, c1:ctok]
            )
    csb = wpool.tile([P, ntiles], dt.float32, tag="csb")
    nc.gpsimd.dma_start(csb[:], cv.rearrange("(t p) -> p t", p=P))
    rssbs = []
    for g in range(nch):
        rssb = wpool.tile([P, ntiles], dt.int32, tag=f"rssb{g}", name=f"rssb{g}")
        nc.gpsimd.dma_start(rssb[:], rscs[g].rearrange("(t p) -> p t", p=P))
        rssbs.append(rssb)
    for f0 in range(8, DF // P, 2):
        nc.gpsimd.dma_start(w2sb[:, f0 : f0 + 2, :], w2r[:, f0 : f0 + 2, :])
    g0sb = wpool.tile([CB, OWN // CB], dt.int32, tag="g0sb")
    nc.gpsimd.dma_start(g0sb[:], g0.rearrange("(t p) -> p t", p=CB))
    g1sb = wpool.tile([CB, OWN // CB], dt.int32, tag="g1sb")
    nc.gpsimd.dma_start(g1sb[:], g1.rearrange("(t p) -> p t", p=CB))

    # ---- interleaved mm1 (2-tile segments) / mm2 (token tiles) ----------
    # mm1 segments are 256 cols: wide enough that ldweights stays hidden
    # under the moving dim, narrow enough that mm2 tiles start early
    segs = []
    t = 0
    while t < ntiles:
        b = min(t + 2, ntiles)
        segs.append((t, b))
        t = b

    fired = [False] * nch
    trig = [None] * nch
    hs = None
    hbase = 0
    done_mm1 = 0
    si = 0

    # k-split phase A for the first KS segments: matmuls over k 0:4 start
    # as soon as HALF of w1 has landed (~12us instead of ~28), drained to
    # SBUF partials on the (idle) Vector engine; phase B adds k 4:8 and
    # runs while the second half of w1 streams in
    KS = min(2, len(segs))
    kh_ = DM // P // 2
    hA = {}
    hapool = ctx.enter_context(tc.tile_pool(name="ha", bufs=1))
    hspool = ctx.enter_context(tc.tile_pool(name="hsum", bufs=3))
    for s in range(KS):
        a, b = segs[s]
        c0, csz = a * P, (b - a) * P
        csz_r = min(csz, tr - c0)
        hA[s] = [
            hapool.tile(
                [P, csz_r], dt.bfloat16, tag=f"hA{s}_{i}", name=f"hA{s}_{i}"
            )
            for i in range(DF // P)
        ]
        for i in range(DF // P):
            ph = phpool.tile([P, csz_r], dt.float32, tag="ph")
            for k in range(kh_):
                nc.tensor.matmul(
                    ph[:],
                    lhsT=w1sb[:, k, i * P : (i + 1) * P],
                    rhs=xsb[:, k, c0 : c0 + csz_r],
                    start=(k == 0),
                    stop=(k == kh_ - 1),
                )
            nc.vector.tensor_scalar_add(hA[s][i][:], ph[:], 0.0)

    def emit_mm1():
        nonlocal hs, hbase, done_mm1, si
        s = si
        a, b = segs[si]
        si += 1
        c0, csz = a * P, (b - a) * P
        # skip the padding columns beyond the real token count: mm2 reads
        # the stale hs region for pad tokens, whose y rows scatter to the
        # garbage row anyway
        csz_r = min(csz, tr - c0)
        hbase = c0
        done_mm1 = c0 + csz
        hs = [
            hpool.tile([P, csz], dt.bfloat16, tag=f"h{i}", name=f"h{i}")
            for i in range(DF // P)
        ]
        k0_ = kh_ if s < KS else 0
        for i in range(DF // P):
            ph = phpool.tile([P, csz_r], dt.float32, tag="ph")
            for k in range(k0_, DM // P):
                nc.tensor.matmul(
                    ph[:],
                    lhsT=w1sb[:, k, i * P : (i + 1) * P],
                    rhs=xsb[:, k, c0 : c0 + csz_r],
                    start=(k == k0_),
                    stop=(k == DM // P - 1),
                )
            if s < KS:
                hsum = hspool.tile([P, csz_r], dt.bfloat16, tag="hsum", name="hsum")
                nc.vector.tensor_add(hsum[:], ph[:], hA[s][i][:])
                nc.scalar.activation(hs[i][:, 0:csz_r], hsum[:], ACT)
            else:
                nc.scalar.activation(hs[i][:, 0:csz_r], ph[:], ACT)

    for tm in range(ntiles):
        while (tm + 1) * P > done_mm1:
            emit_mm1()
        toff = tm * P - hbase
        py0 = pypool.tile([P, 512], dt.float32, tag="py")
        py1 = pypool.tile([P, 512], dt.float32, tag="py")
        for f in range(DF // P):
            lhs = hs[f][:, toff : toff + P]
            nc.tensor.matmul(
                py0[:], lhsT=lhs, rhs=w2sb[:, f, 0:512],
                start=(f == 0), stop=(f == DF // P - 1),
            )
            nc.tensor.matmul(
                py1[:], lhsT=lhs, rhs=w2sb[:, f, 512:1024],
                start=(f == 0), stop=(f == DF // P - 1),
            )
        # scale on the (otherwise idle) Scalar engine: keeping these off the
        # Vector queue stops the combine adds' gather-waits from head-blocking
        # the PSUM release chain
        y_sb = ypool.tile([P, DM], dt.bfloat16, tag="y")
        nc.scalar.activation(
            y_sb[:, 0:512], py0[:],
            mybir.ActivationFunctionType.Copy, scale=csb[:, tm : tm + 1],
        )
        nc.scalar.activation(
            y_sb[:, 512:1024], py1[:],
            mybir.ActivationFunctionType.Copy, scale=csb[:, tm : tm + 1],
        )
        # scatter only the real rows (pad rows would land on the garbage
        # row anyway). Split by COLUMN ranges via element_offset -- pieces
        # stream on separate DMA queues (~23 GB/s each), cutting the
        # y->send latency between the last mm2 tile and its A2A doorbell.
        # (All partition bases stay 0: base>0 indirect operands crash the
        # DMA ucode.)
        rows = min(P, tr - tm * P)
        # gating chunks (whose A2A fires after this tile) scatter FIRST
        # and trigger immediately; other chunks' scatters queue after the
        # doorbell so they don't delay it
        for g in sorted(gsets[tm], key=lambda g_: Ts[g_] != tm + 1):
            gating = Ts[g] == tm + 1
            nsp = 2 if SPLIT_SCATTER else 1
            splits = [(i * DM // nsp, (i + 1) * DM // nsp) for i in range(nsp)]
            for e0, e1 in splits:
                last_scat = nc.gpsimd.indirect_dma_start(
                    out=sends[g][:],
                    out_offset=bass.IndirectOffsetOnAxis(
                        ap=rssbs[g][0:rows, tm : tm + 1], axis=0
                    ),
                    in_=y_sb[0:rows, e0:e1],
                    in_offset=None,
                    element_offset=e0,
                )
            if gating and not fired[g]:
                trig[g] = nc.gpsimd.collective_compute(
                    "AllToAll",
                    mybir.AluOpType.bypass,
                    replica_groups=groups,
                    ins=[sends[g][0 : NCORES * sg[g], :]],
                    outs=[recvbuf[Rg[g] : Rg[g + 1], :]],
                )
                fired[g] = True
        for g in range(nch):
            if not fired[g] and Ts[g] == tm + 1:
                trig[g] = nc.gpsimd.collective_compute(
                    "AllToAll",
                    mybir.AluOpType.bypass,
                    replica_groups=groups,
                    ins=[sends[g][0 : NCORES * sg[g], :]],
                    outs=[recvbuf[Rg[g] : Rg[g + 1], :]],
                )
                fired[g] = True
    assert all(fired), (Ts, ntiles)

    # ---- combine: class-sorted 64-token tiles, prefix-sliced gathers ----
    # per-token rows are host-ordered (g0 = earlier-landing chunk, g1 =
    # later), each gather gated/prefixed by its own class so the early-row
    # gather overlaps the last A2A chunk instead of waiting for it
    for j in range(OWN // CB):
        last_cls = comb_cls1[j] == nch - 1
        ga = gpool.tile([CB, DM], dt.bfloat16, tag="ga")
        gb = gpool.tile([CB, DM], dt.bfloat16, tag="gb")
        gis = []
        # last-class gathers are on the post-final-A2A critical path: split
        # them by column half (element_offset) so the two 64KB pieces
        # stream on two DMA queues
        halves = (
            ((0, 512), (512, DM)) if (last_cls and SPLIT_GATHER) else ((0, DM),)
        )
        for buf, gsb, cls in ((ga, g0sb, comb_cls0[j]), (gb, g1sb, comb_cls1[j])):
            for e0, e1 in halves:
                gi = nc.gpsimd.indirect_dma_start(
                    out=buf[:, e0:e1],
                    out_offset=None,
                    in_=recvbuf[0 : Rg[cls + 1], :],
                    in_offset=bass.IndirectOffsetOnAxis(
                        ap=gsb[:, j : j + 1], axis=0
                    ),
                    element_offset=e0,
                )
                gis.append((gi, cls))
        # queue class-c gathers behind the DOORBELL of chunk c+1: the
        # scheduler otherwise slots them between scatters and a trigger,
        # where their (blocked) recvbuf wait head-blocks the gpsimd queue
        # and delays the doorbell until the previous chunk lands. Gating on
        # c+1's trigger still lets early-class gathers overlap later chunks.
        for gi, c in gis:
            gate = trig[min(c + 1, nch - 1)]
            add_dep_helper(gi.ins, gate.ins, sync=False, reason="doorbells first")
        ys = gpool.tile([CB, DM], dt.bfloat16, tag="ys")
        nc.vector.tensor_add(ys[:], ga[:], gb[:])
        # split the output write across two issue queues (two DMA queues)
        nc.sync.dma_start(yout[j * CB : j * CB + CB // 2, :], ys[0 : CB // 2, :])
        nc.scalar.dma_start(yout[j * CB + CB // 2 : (j + 1) * CB, :], ys[CB // 2 : CB, :])


def _build_program(cfg):
    key = cfg[:8]
    if key in _PROGRAM_CACHE:
        return _PROGRAM_CACHE[key]
    from contextlib import ExitStack

    nc = bacc.Bacc(
        "TRN2",
        target_bir_lowering=False,
        debug=False,
        enable_asserts=True,
        num_devices=NCORES,
    )
    with tile.TileContext(nc) as tc:
        with ExitStack() as ctx:
            _emit(nc, tc, ctx, cfg)
    nc.compile()
    _PROGRAM_CACHE[key] = nc
    return nc


def _prepare(x, topk_e, topk_w):
    """Host-side routing: dispatch tokens to experts.

    Layout: token with slot s in (expert e -> owner d) block, s in
    [B_g, B_{g+1}):  send-side row (core e, tensor send_g) = d*sg + (s-B_g);
    recv-side row (core d, recvbuf) = 8*B_g + e*sg + (s-B_g).
    """
    bf16 = ml_dtypes.bfloat16
    c = np.zeros((S, E), dtype=np.float32)
    np.add.at(c, (np.arange(S)[:, None], topk_e), topk_w.astype(np.float32))
    single = topk_e[:, 0] == topk_e[:, 1]

    toks = [np.nonzero((topk_e == e).any(axis=1))[0] for e in range(E)]
    cnt = np.zeros((E, NCORES), dtype=np.int64)
    for e in range(E):
        cnt[e] = np.bincount(toks[e] // OWN, minlength=NCORES)
    blk = int(cnt.max())

    ntok_max = max(len(t) for t in toks)
    ctok = int(-(-ntok_max // P) * P)
    ntiles = ctok // P

    # Adaptive chunk boundaries: chunk g should trigger after tile
    # targets[g], i.e. the largest 16-multiple slot b such that every
    # expert's token count with slot < b fits in targets[g]*128 rows.
    # Late triggers get small chunks so the post-compute tail is short.
    def maxcum(b):
        return int(np.minimum(cnt, b).sum(axis=1).max())

    targets = [ntiles // 2, ntiles - 2, ntiles - 1]
    B = [0]
    for tgt in targets:
        b = B[-1]
        nb = b
        while nb + 16 < blk and maxcum(nb + 16) <= tgt * P:
            nb += 16
        if nb > b:
            B.append(nb)
    B.append(blk)
    B = sorted(set(B))
    nch = len(B) - 1
    Ba = np.array(B)
    sga = np.diff(Ba)
    Rga = NCORES * Ba

    in_maps = []
    row_of = {}  # (e, token) -> absolute recvbuf row (on the owner core)
    cums = np.zeros((E, nch), dtype=np.int64)
    gsets = [set() for _ in range(ntiles)]
    for e in range(E):
        te = toks[e]
        d = te // OWN
        seg_start = np.searchsorted(te, np.arange(NCORES) * OWN)
        slot = np.arange(len(te)) - seg_start[d]
        gi = np.searchsorted(Ba[1:-1], slot, side="right")
        srow = d * sga[gi] + (slot - Ba[gi])  # send side, relative to send_g
        rrow = Rga[gi] + e * sga[gi] + (slot - Ba[gi])  # recv side, absolute
        for t, r in zip(te, rrow):
            row_of[(e, int(t))] = int(r)
        order = np.lexsort((slot, d, gi))
        te_o = te[order]
        gi_o = np.full(ctok, nch - 1, dtype=np.int64)
        gi_o[: len(te)] = gi[order]
        xT_e = np.zeros((DM, ctok), dtype=bf16)
        xT_e[:, : len(te)] = x[te_o].T.astype(bf16)
        cv_e = np.zeros(ctok, dtype=np.float32)
        w = c[te_o, e]
        cv_e[: len(te)] = np.where(single[te_o], 0.5 * w, w)
        im = {"xT": xT_e, "cv": cv_e}
        srow_o = np.zeros(ctok, dtype=np.int64)
        srow_o[: len(te)] = srow[order]
        for g in range(nch):
            rs = np.full(ctok, NCORES * sga[g], dtype=np.int32)  # garbage row
            sel = gi_o == g
            sel[len(te) :] = False
            rs[sel] = srow_o[sel]
            im[f"rsc{g}"] = rs
            cums[e, g] = int(np.sum(gi <= g))
        for tm in range(ntiles):
            for g in np.unique(gi_o[tm * P : (tm + 1) * P]):
                gsets[tm].add(int(g))
        in_maps.append(im)

    Ts = [min(int(np.ceil(cums[:, g].max() / P)), ntiles) for g in range(nch)]
    for g in range(1, nch):
        Ts[g] = max(Ts[g], Ts[g - 1])
    Ts[-1] = ntiles

    chunk_of_row = lambda r: int(np.searchsorted(Rga[1:], r, side="right"))
    perms = []
    comb_cls0 = np.zeros((NCORES, OWN // CB), dtype=np.int64)
    comb_cls1 = np.zeros((NCORES, OWN // CB), dtype=np.int64)
    for dcore in range(NCORES):
        r0a = np.zeros(OWN, dtype=np.int32)
        r1a = np.zeros(OWN, dtype=np.int32)
        cls0 = np.zeros(OWN, dtype=np.int64)
        cls1 = np.zeros(OWN, dtype=np.int64)
        for t_loc in range(OWN):
            t = dcore * OWN + t_loc
            es = np.unique(topk_e[t])
            ra = row_of[(int(es[0]), t)]
            rb = row_of[(int(es[1]), t)] if len(es) > 1 else ra
            ca, cb = chunk_of_row(ra), chunk_of_row(rb)
            if ca > cb:
                ra, rb, ca, cb = rb, ra, cb, ca
            r0a[t_loc], r1a[t_loc] = ra, rb
            cls0[t_loc], cls1[t_loc] = ca, cb
        perm = np.argsort(cls1, kind="stable")
        perms.append(perm)
        comb_cls0[dcore] = cls0[perm].reshape(OWN // CB, CB).max(axis=1)
        comb_cls1[dcore] = cls1[perm].reshape(OWN // CB, CB).max(axis=1)
        in_maps[dcore]["g0"] = r0a[perm]
        in_maps[dcore]["g1"] = r1a[perm]

    cfg = (
        blk,
        tuple(B),
        ctok,
        ntok_max,
        tuple(Ts),
        tuple(int(v) for v in comb_cls0.max(axis=0)),
        tuple(int(v) for v in comb_cls1.max(axis=0)),
        tuple(tuple(sorted(s)) for s in gsets),
        tuple(tuple(int(v) for v in p) for p in perms),
    )
    return in_maps, cfg


def prepare_in_maps(x, topk_e, topk_w, w1, w2):
    bf16 = ml_dtypes.bfloat16
    in_maps, cfg = _prepare(np.asarray(x), np.asarray(topk_e), np.asarray(topk_w))
    for e in range(E):
        in_maps[e]["w1t"] = np.ascontiguousarray(np.asarray(w1)[e].T).astype(bf16)
        in_maps[e]["w2t"] = np.ascontiguousarray(np.asarray(w2)[e].T).astype(bf16)
    return in_maps, cfg


def postprocess(results, cfg):
    perms = cfg[8]
    out = np.empty((S, DM), dtype=np.float32)
    for d in range(NCORES):
        out[d * OWN + np.asarray(perms[d], dtype=np.int64)] = results[d][
            "yout"
        ].astype(np.float32)
    return out


def kernel(x, topk_e, topk_w, w1, w2):
    in_maps, cfg = prepare_in_maps(x, topk_e, topk_w, w1, w2)
    nc = _build_program(cfg)
    res = run_bass_kernel_spmd(nc, in_maps, list(range(NCORES)))
    return postprocess(res.results, cfg)
